# revision 3
# baseline (speedup 1.0000x reference)
"""Trainium2 Bass kernel for nn_CNOLReLu: bicubic 2x upsample -> leaky_relu
-> antialiased bicubic 2x downsample on a (16,128,128,128) NHWC tensor.

Strategy (per NeuronCore, 2 images each, data-parallel over batch):
  Per channel c the op is  Y = D @ f(U @ X @ U.T) @ D.T  with X = x[b,:,:,c],
  U = 128->256 bicubic matrix, D = 256->128 antialiased bicubic matrix,
  f = leaky_relu(0.01).  Four matmul hops on the tensor engine (ping-pong
  layouts so no transposes are needed):
    A: pA = X^T  @ U^T          [w,  h2]  (fp32r, lhsT = X column-slice)
    B: pZ = U    @ pA           [w2, h2]  (fp32r, lhsT = U^T chunk)
    f: Lrelu on ScalarE during PSUM->SBUF evac (casts to bf16)
    C: pS = f(Z)^T chunk @ D^T  [h2, w']  (bf16, banded: D is 8-tap)
    D: pY = D    @ pS           [h', w']  (fp32r, 4 channels packed, N=512)
"""
import numpy as np
import ml_dtypes
from contextlib import ExitStack

import concourse.bacc as bacc
import concourse.tile as tile
from concourse import mybir
from concourse.bass_utils import run_bass_kernel_spmd
import concourse.bass as bass

F32 = mybir.dt.float32
F32R = mybir.dt.float32r
BF16 = mybir.dt.bfloat16
AF = mybir.ActivationFunctionType

N_CORES = 8
B_CORE = 2          # images per core
H = W = C = 128
H2 = W2 = 256
NEG_SLOPE = 0.01


def _keys_cubic(x):
    x = np.abs(x)
    return np.where(
        x <= 1, (1.5 * x - 2.5) * x * x + 1,
        np.where(x < 2, ((-0.5 * x + 2.5) * x - 4) * x + 2, 0.0))


def _resize_matrix(n_in, n_out):
    """Row-stochastic bicubic (antialias) resize operator, matches
    jax.image.resize(method='bicubic', antialias=True)."""
    scale = n_out / n_in
    pos = (np.arange(n_out) + 0.5) / scale - 0.5
    kscale = min(scale, 1.0)
    w = _keys_cubic((np.arange(n_in)[None, :] - pos[:, None]) * kscale)
    return (w / w.sum(axis=1, keepdims=True)).astype(np.float64)


def _band(Dm, t):
    """Output rows of D touched by input-column chunk t (128 cols)."""
    rows = np.nonzero(np.abs(Dm[:, t * 128:(t + 1) * 128]).sum(1) > 0)[0]
    return int(rows.min()), int(rows.max()) + 1


_CACHE = {}


def _build():
    if "nc" in _CACHE:
        return _CACHE["nc"], _CACHE["consts"]

    U = _resize_matrix(H, H2)          # [256,128]
    Dm = _resize_matrix(H2, H)         # [128,256]
    uT = np.ascontiguousarray(U.T).astype(np.float32)                # [128,256]
    dT = np.concatenate([Dm.T[0:128, :], Dm.T[128:256, :]], axis=1)  # [128,256]
    dT_bf = dT.astype(ml_dtypes.bfloat16)
    dT_r = dT.astype(np.float32)
    bands = [_band(Dm, 0), _band(Dm, 1)]   # [(0,66),(62,128)]

    nc = bacc.Bacc()
    x_d = nc.declare_dram_parameter("x", [B_CORE, H, W, C], F32R, isOutput=False)
    ut_d = nc.declare_dram_parameter("ut", [128, 256], F32R, isOutput=False)
    dbf_d = nc.declare_dram_parameter("dbf", [128, 256], BF16, isOutput=False)
    dr_d = nc.declare_dram_parameter("dr", [128, 256], F32R, isOutput=False)
    y_d = nc.declare_dram_parameter("y", [B_CORE, H, W, C], F32, isOutput=True)

    with tile.TileContext(nc) as tc, ExitStack() as ctx:
        wpool = ctx.enter_context(tc.tile_pool(name="weights", bufs=1))
        xpool = ctx.enter_context(tc.tile_pool(name="ximg", bufs=1))
        opool = ctx.enter_context(tc.tile_pool(name="oimg", bufs=1))
        spool = ctx.enter_context(tc.tile_pool(name="stage", bufs=2))
        sapool = ctx.enter_context(tc.tile_pool(name="fine", bufs=4))
        ppool = ctx.enter_context(tc.tile_pool(name="psum", bufs=2, space="PSUM"))

        ut_s = wpool.tile([128, 256], F32R, tag="ut")
        dbf_s = wpool.tile([128, 256], BF16, tag="dbf")
        dr_s = wpool.tile([128, 256], F32R, tag="dr")
        nc.sync.dma_start(ut_s[:], ut_d[:])
        nc.sync.dma_start(dbf_s[:], dbf_d[:])
        nc.sync.dma_start(dr_s[:], dr_d[:])

        for b in range(B_CORE):
            ximg = xpool.tile([128, W * C], F32R, tag="ximg")
            nc.sync.dma_start(ximg[:], x_d[b].rearrange("h w c -> h (w c)"))
            oimg = opool.tile([128, W * C], F32, tag="oimg")

            for g in range(C // 4):          # 4-channel groups
                sS = spool.tile([128, 1024], F32R, tag="sS")
                pY = ppool.tile([128, 512], F32, tag="pY")
                for p in range(2):           # channel pairs in group
                    c0 = g * 4 + p * 2
                    # ---- A: per channel, pA[:, c*256:(c+1)*256] = X_c^T U^T
                    pA = ppool.tile([128, 512], F32, tag="pA")
                    for ci in range(2):
                        c = c0 + ci
                        xc = ximg[:, c::C]            # [h, w] stride-C view
                        nc.tensor.matmul(pA[:, ci * 256:(ci + 1) * 256],
                                         xc, ut_s[:], start=True, stop=True)
                    sP = spool.tile([128, 512], F32R, tag="sP")
                    nc.vector.tensor_copy(sP[:], pA[:])

                    # ---- B: pZ_t = U_chunk_t @ sP   (both channels at once)
                    # sP free layout: (ci, h2); pZ free layout: (ci, h2)
                    sA = sapool.tile([128, 1024], BF16, tag="sA")
                    for t in range(2):
                        pZ = ppool.tile([128, 512], F32, tag="pZ")
                        nc.tensor.matmul(pZ[:], ut_s[:, t * 128:(t + 1) * 128],
                                         sP[:], start=True, stop=True)
                        # ---- leaky relu (exact) + cast to bf16 on ScalarE
                        nc.scalar.activation(sA[:, t * 512:(t + 1) * 512],
                                             pZ[:], AF.Lrelu, alpha=NEG_SLOPE)

                    # ---- C: banded W-down, bf16.
                    # pS free layout: (ci, m, w'): col = ci*256 + m*128 + w'
                    pS = ppool.tile([128, 512], F32, tag="pS")
                    for ci in range(2):
                        for m in range(2):
                            for t in range(2):
                                lo, hi = bands[t]
                                nc.tensor.matmul(
                                    pS[:, ci * 256 + m * 128 + lo:
                                       ci * 256 + m * 128 + hi],
                                    sA[:, t * 512 + ci * 256 + m * 128:
                                       t * 512 + ci * 256 + (m + 1) * 128],
                                    dbf_s[:, t * 128 + lo:t * 128 + hi],
                                    start=(t == 0), stop=(t == 1),
                                    skip_group_check=True)
                    # ---- evac pS -> sS with (m, w', c4) layout:
                    # dst col = m*512 + w*4 + (2*p + ci); src col = ci*256+m*128+w
                    src = pS[:].rearrange("q (ci m w) -> q ci m w", ci=2, m=2)
                    dst = sS[:].rearrange("q (m w c) -> q m w c", m=2, w=128)
                    for ci in range(2):
                        nc.vector.tensor_copy(
                            dst[:, :, :, 2 * p + ci], src[:, ci])

                # ---- D: pY[h', (w',c4)] = sum_m D_chunk_m @ sS_m  (fp32r N=512)
                for m in range(2):
                    nc.tensor.matmul(pY[:], dr_s[:, m * 128:(m + 1) * 128],
                                     sS[:, m * 512:(m + 1) * 512],
                                     start=(m == 0), stop=(m == 1))
                # ---- evac pY -> oimg columns w'*C + c for c in group
                dsto = oimg[:].rearrange("h (w c) -> h w c", c=128)[:, :, g * 4:(g + 1) * 4]
                srco = pY[:].rearrange("h (w c) -> h w c", c=4)
                nc.scalar.copy(dsto, srco)

            nc.sync.dma_start(y_d[b].rearrange("h w c -> h (w c)"), oimg[:])

    nc.compile()
    consts = {"ut": uT, "dbf": np.ascontiguousarray(dT_bf),
              "dr": np.ascontiguousarray(dT_r)}
    _CACHE["nc"] = nc
    _CACHE["consts"] = consts
    return nc, consts


def kernel(x, in_size=128, out_size=128, trace=False):
    x = np.asarray(x, dtype=np.float32)
    assert x.shape == (16, H, W, C), x.shape
    nc, consts = _build()
    in_maps = []
    for core in range(N_CORES):
        m = {"x": np.ascontiguousarray(x[core * B_CORE:(core + 1) * B_CORE])}
        m.update(consts)
        in_maps.append(m)
    res = run_bass_kernel_spmd(nc, in_maps, list(range(N_CORES)), trace=trace)
    out = np.concatenate([res.results[i]["y"] for i in range(N_CORES)], axis=0)
    if trace:
        kernel.last_exec_time_ns = res.exec_time_ns
        kernel.last_results = res
    return out.astype(np.float32)
